# revision 12
# baseline (speedup 1.0000x reference)
"""Local causal (sliding-window) attention kernel for Trainium2, SPMD over 8 cores.

Problem: states [4, 4096, 1024] f32; q/k/v = states @ W*.T + b*; each query t
attends keys t-8..t (window=8), softmax over valid positions, out = attn @ v.

Sharding: data-parallel, 8 shards = 4 batches x 2 sequence halves (2048 queries
each). Each shard's states arrive pre-transposed and chunk-packed as
[128, 8, 2056] with an 8-col halo at the sequence start (zeros for the first
half; real previous-half tokens for the second half).

Score reformulation (saves one full GEMM): q.k = x_t^T A x_k + u[k] + const
with A = (Wq/sqrt(H))^T Wk precomputed on host. The device computes
Y = A @ X (one GEMM); scores come out TRANSPOSED (keys on partitions):
S^T_i = Y[:, frame_i]^T @ X[:, queries_i], which feeds softmax along the
partition dim with no transposes: exp bias = u[key] (per-partition), band
mask applied multiplicatively after exp, row-sum via a PE matmul against a
ones column, and P^T is directly the PV lhsT.

Tiling: 17 full tiles of 120 queries + 1 tail tile of 8. Each 120-query tile's
9-key windows span exactly 128 keys -> one sliding V frame per tile, so PV is
2 matmuls (plus the N=1 rowsum). V frames are recomputed on the 8-col overlap
(+6% V GEMM); the tail tile's 16-key V frame comes from the host (vtail).

Schedule: PE warm-up dummies during the initial weight DMA (HAM), Y GEMM
first (5 chunks, first small to shorten the DMA critical path), then a
per-frame software pipeline V_i | S^T_i | PV_{i-1} so the tail is one chain.
Inputs ride two HWDGE rings (x on sync, a/wv on scalar); outputs on gpsimd.
"""

import numpy as np
import ml_dtypes

import concourse.bacc as bacc
import concourse.mybir as mybir
import concourse.tile as tile
from concourse.bass_utils import run_bass_kernel_spmd

B, T, H = 4, 4096, 1024
NCORES = 8
TC = T // 2            # queries per core
HALO = 8               # window size
TH = TC + HALO         # x cols incl. halo
QT = 120               # queries per full tile (window spans exactly 128 keys)
NFT = 17               # full tiles; tail tile has TC - 17*120 = 8 queries
NTILE = NFT + 1
NQ_TAIL = TC - QT * NFT            # 8
NK_TAIL = NQ_TAIL + HALO           # 16
HC = H // 128          # 128-row chunks of H
NWARM = 26             # HAM warm-up dummy matmuls
# Y GEMM x-col chunks; first small so PE can start after ~2.5MB of DMA
YCHUNKS = [(8, 264), (264, 776), (776, 1288), (1288, 1800), (1800, 2056)]
XSEGS = [(0, 264), (264, 776), (776, 1288), (1288, 1800), (1800, 2056)]
F32 = mybir.dt.float32
BF16 = mybir.dt.bfloat16
BF = ml_dtypes.bfloat16
AF = mybir.ActivationFunctionType

_cache = {}


def _emit(nc, tc, aps, pools):
    (xs_d, a_d, wv_d, bv_d, m0_d, mr_d, u2_d, yh_d, vt_d, out_d) = aps
    consts, xw, acts, attn, psY, psS, psO, psR = pools

    warm = consts.tile([128, 512], BF16, tag="warm", name="warm")
    ones_t = consts.tile([128, 1], BF16, tag="ones", name="ones_t")
    bv_t = consts.tile([128, H], BF16, tag="bv", name="bv_t")
    m0_t = consts.tile([128, QT], BF16, tag="m0", name="m0_t")
    mr_t = consts.tile([128, QT], BF16, tag="mr", name="mr_t")
    u2_t = consts.tile([128, NTILE], F32, tag="u2", name="u2_t")
    vtail_t = consts.tile([NK_TAIL, H], BF16, tag="vtail", name="vtail_t")

    x_all = xw.tile([128, HC, TH], BF16, tag="x", name="x_all")
    a_all = xw.tile([128, HC, H], BF16, tag="a", name="a_all")
    wv_all = xw.tile([128, HC, H], BF16, tag="wv", name="wv_all")
    y_all = acts.tile([128, HC, TH], BF16, tag="y", name="y_all")
    vt = [acts.tile([128, H], BF16, tag=f"v{i}", name=f"v{i}")
          for i in range(NFT)]

    # ---- DMA issue. ALL inputs ride one HWDGE ring (scalar) in exact
    # need-order so the critical load (xseg0+a) gets the full HBM bandwidth;
    # outputs go on the sync ring, which is idle until the attention phase.
    def xdma(s):
        lo, hi = XSEGS[s]
        nc.scalar.dma_start(x_all[:, :, lo:hi], xs_d[s][:])
    xdma(0)
    nc.scalar.dma_start(a_all[:], a_d[:])
    xdma(1)
    xdma(2)
    xdma(3)
    xdma(4)
    nc.scalar.dma_start(wv_all[:], wv_d[:])
    nc.scalar.dma_start(y_all[:, :, 0:HALO], yh_d[:])
    nc.scalar.dma_start(u2_t[:], u2_d[:])
    nc.scalar.dma_start(vtail_t[:], vt_d[:])
    nc.scalar.dma_start(bv_t[:], bv_d[:])
    nc.scalar.dma_start(m0_t[:], m0_d[:])
    nc.scalar.dma_start(mr_t[:], mr_d[:])

    # ---- PE warm-up on a zeroed tile while weights stream in (HAM) ----
    nc.vector.memset(warm[:], 0.0)
    nc.vector.memset(ones_t[:], 1.0)
    for _ in range(NWARM):
        ps = psY.tile([128, 512], F32, tag="ps", name="ps_warm")
        nc.tensor.matmul(ps[:], warm[:, 0:128], warm[:], start=True, stop=True)

    # ---- Y = A @ X over all x cols (halo cols from host) ----
    for ci, (lo, hi) in enumerate(YCHUNKS):
        for hc in range(HC):
            ps = psY.tile([128, hi - lo], F32, tag="ps", name="ps_y")
            for c in range(HC):
                nc.tensor.matmul(ps[:], a_all[:, c, hc * 128:(hc + 1) * 128],
                                 x_all[:, c, lo:hi],
                                 start=(c == 0), stop=(c == HC - 1))
            if hc % 2 == 0:
                nc.scalar.copy(y_all[:, hc, lo:hi], ps[:])
            else:
                nc.vector.tensor_copy(y_all[:, hc, lo:hi], ps[:])

    # ---- V frames + attention, software-pipelined per frame ----
    pm_tiles = {}
    rq = [QT] * NFT + [NQ_TAIL]
    rk = [128] * NFT + [NK_TAIL]

    def emit_v(i):
        for hh in range(2):
            ps = psY.tile([128, 512], F32, tag="ps", name="ps_v")
            for c in range(HC):
                nc.tensor.matmul(ps[:], x_all[:, c, QT * i: QT * i + 128],
                                 wv_all[:, c, hh * 512:(hh + 1) * 512],
                                 start=(c == 0), stop=(c == HC - 1))
            nc.vector.tensor_add(vt[i][:, hh * 512:(hh + 1) * 512], ps[:],
                                 bv_t[:, hh * 512:(hh + 1) * 512])

    def emit_s(i):
        nq, nk, f0 = rq[i], rk[i], QT * i
        s_ps = psS.tile([128, QT], F32, tag="s", name="s_ps")
        for c in range(HC):
            nc.tensor.matmul(s_ps[:nk, :nq], y_all[:, c, f0:f0 + nk],
                             x_all[:, c, f0 + HALO:f0 + HALO + nq],
                             start=(c == 0), stop=(c == HC - 1))
        p = attn.tile([128, QT], BF16, tag="p", name="p")
        nc.scalar.activation(p[:nk, :nq], s_ps[:nk, :nq], AF.Exp,
                             bias=u2_t[0:nk, i:i + 1], scale=1.0)
        pm = attn.tile([128, QT], BF16, tag="pm", name="pm")
        mask = m0_t if i == 0 else mr_t
        nc.vector.tensor_mul(pm[:nk, :nq], p[:nk, :nq], mask[0:nk, 0:nq])
        pm_tiles[i] = pm

    def emit_pv(i):
        nq, nk = rq[i], rk[i]
        pm = pm_tiles.pop(i)
        vsrc = vt[i] if i < NFT else vtail_t
        rs_ps = psR.tile([QT, 1], F32, tag="rs", name="rs_ps")
        nc.tensor.matmul(rs_ps[:nq, :], pm[:nk, :nq], ones_t[0:nk, :],
                         start=True, stop=True)
        rinv = attn.tile([QT, 1], F32, tag="ri", name="rinv")
        nc.vector.reciprocal(rinv[:nq, :], rs_ps[:nq, :])
        out_sb = attn.tile([QT, H], F32, tag="osb", name="out_sb")
        for hh in range(2):
            o_ps = psO.tile([QT, 512], F32, tag="o", name="o_ps")
            nc.tensor.matmul(o_ps[:nq, :], pm[:nk, :nq],
                             vsrc[0:nk, hh * 512:(hh + 1) * 512],
                             start=True, stop=True)
            if hh == 0:
                nc.scalar.activation(out_sb[:nq, 0:512], o_ps[:nq, :],
                                     AF.Copy, bias=0.0, scale=rinv[:nq, :])
                nc.sync.dma_start(out_d[QT * i: QT * i + nq, 0:512],
                                  out_sb[:nq, 0:512])
            else:
                nc.vector.tensor_scalar_mul(out_sb[:nq, 512:H], o_ps[:nq, :],
                                            rinv[:nq, :])
                nc.sync.dma_start(out_d[QT * i: QT * i + nq, 512:H],
                                  out_sb[:nq, 512:H])

    # Tail tile first: it needs only Y + vtail (no V frame), so its whole
    # chain clears early and the kernel tail is just tile 16's chain.
    emit_s(NTILE - 1)
    emit_pv(NTILE - 1)
    for i in range(NFT):
        emit_v(i)
        emit_s(i)
        if i >= 1:
            emit_pv(i - 1)
    emit_pv(NFT - 1)


def _build(loop_reps=None, trace_sim=False):
    key = ("nc", loop_reps, trace_sim)
    if key in _cache:
        return _cache[key]
    nc = bacc.Bacc("TRN2", target_bir_lowering=False, debug=False,
                   num_devices=NCORES)

    aps = (
        [nc.dram_tensor(f"x{s}", [128, HC, hi - lo], BF16,
                        kind="ExternalInput").ap()
         for s, (lo, hi) in enumerate(XSEGS)],
        nc.dram_tensor("a", [128, HC, H], BF16, kind="ExternalInput").ap(),
        nc.dram_tensor("wv", [128, HC, H], BF16, kind="ExternalInput").ap(),
        nc.dram_tensor("bv", [128, H], BF16, kind="ExternalInput").ap(),
        nc.dram_tensor("m0", [128, QT], BF16, kind="ExternalInput").ap(),
        nc.dram_tensor("mr", [128, QT], BF16, kind="ExternalInput").ap(),
        nc.dram_tensor("u2", [128, NTILE], F32, kind="ExternalInput").ap(),
        nc.dram_tensor("yhalo", [128, HC, HALO], BF16,
                       kind="ExternalInput").ap(),
        nc.dram_tensor("vtail", [NK_TAIL, H], BF16, kind="ExternalInput").ap(),
        nc.dram_tensor("out", [TC, H], F32, kind="ExternalOutput").ap(),
    )

    with tile.TileContext(nc, trace_sim=trace_sim) as tc:
        with (
            tc.tile_pool(name="consts", bufs=1) as consts,
            tc.tile_pool(name="xw", bufs=1) as xw,
            tc.tile_pool(name="acts", bufs=1) as acts,
            tc.tile_pool(name="attn", bufs=3) as attn,
            tc.tile_pool(name="psY", bufs=3, space="PSUM") as psY,
            tc.tile_pool(name="psS", bufs=2, space="PSUM") as psS,
            tc.tile_pool(name="psO", bufs=2, space="PSUM") as psO,
            tc.tile_pool(name="psR", bufs=1, space="PSUM") as psR,
        ):
            pools = (consts, xw, acts, attn, psY, psS, psO, psR)
            if loop_reps:
                with tc.For_i(0, loop_reps, 1):
                    _emit(nc, tc, aps, pools)
            else:
                _emit(nc, tc, aps, pools)

    nc.compile()
    _cache[key] = nc
    return nc


def _pack(m):
    """[128*HC, W] row-chunked -> [128, HC, W] (partition-major packing)."""
    w = m.shape[1]
    return np.ascontiguousarray(
        m.reshape(HC, 128, w).transpose(1, 0, 2))


def _host_inputs(states, Wq, bq, Wk, bk, Wv, bv):
    """Shared (per-run) host-side tensor prep."""
    scale = 1.0 / np.sqrt(H)
    Wq = np.asarray(Wq, np.float32)
    Wk = np.asarray(Wk, np.float32)
    Wv = np.asarray(Wv, np.float32)
    bq = np.asarray(bq, np.float32)
    bv = np.asarray(bv, np.float32)
    Wqs = Wq * scale
    # A = Wqs.T @ Wk ; device lhsT layout needs A.T = Wk.T @ Wqs
    at_h = np.ascontiguousarray(Wk.T @ Wqs).astype(BF)
    # per-key rank-1 vector; per-query term and constants cancel in softmax
    wt_h = Wk.T @ (bq * scale)
    wv_h = np.ascontiguousarray(Wv.T).astype(BF)
    a_p = _pack(at_h.astype(BF))
    wv_p = _pack(wv_h.astype(BF))
    bv_h = np.ascontiguousarray(np.broadcast_to(bv, (128, H))).astype(BF)
    k = np.arange(128)[:, None]
    t = np.arange(QT)[None, :]
    band = (k >= t) & (k <= t + HALO)
    mr_h = band.astype(BF)
    m0_h = (band & (k >= HALO)).astype(BF)
    return at_h, wt_h, wv_p, bv_h, m0_h, mr_h, a_p, bv


def _shard_maps(states, hosts):
    at_h, wt_h, wv_p, bv_h, m0_h, mr_h, a_p, bv = hosts
    a_f = at_h.astype(np.float32)      # A.T in bf16 precision
    wv_f = wv_p.transpose(1, 0, 2).reshape(H, H).astype(np.float32)  # Wv.T
    in_maps = []
    for i in range(NCORES):
        b, hf = i // 2, i % 2
        xs = np.zeros((TH, H), np.float32)
        if hf == 0:
            xs[HALO:] = states[b, 0:TC]
        else:
            xs[:] = states[b, TC - HALO: 2 * TC]
        x_h = np.ascontiguousarray(xs.T).astype(BF)   # [H, TH]
        x_f = x_h.astype(np.float32)
        u_full = wt_h @ x_f                            # [TH]
        u2 = np.zeros((128, NTILE), np.float32)
        for j in range(NFT):
            u2[:, j] = u_full[QT * j: QT * j + 128]
        u2[:NK_TAIL, NFT] = u_full[QT * NFT: QT * NFT + NK_TAIL]
        yh = (a_f.T @ x_f[:, :HALO])                   # [H, 8] = A @ x_halo
        vtail_h = (x_f[:, QT * NFT:].T @ wv_f + bv).astype(BF)  # [16, H]
        im = {
            "a": a_p, "wv": wv_p, "bv": bv_h,
            "m0": (m0_h if hf == 0 else mr_h), "mr": mr_h,
            "u2": u2, "yhalo": _pack(yh.astype(BF)), "vtail": vtail_h,
        }
        for s, (lo, hi) in enumerate(XSEGS):
            im[f"x{s}"] = _pack(x_h[:, lo:hi])
        in_maps.append(im)
    return in_maps


def kernel(states, Wq, bq, Wk, bk, Wv, bv, window):
    assert int(window) == HALO
    states = np.asarray(states, np.float32)
    nc = _build()
    hosts = _host_inputs(states, Wq, bq, Wk, bk, Wv, bv)
    in_maps = _shard_maps(states, hosts)
    res = run_bass_kernel_spmd(nc, in_maps, list(range(NCORES)))
    out = np.empty((B, T, H), np.float32)
    for i in range(NCORES):
        b, hf = i // 2, i % 2
        out[b, hf * TC:(hf + 1) * TC] = res.results[i]["out"]
    return out


# revision 14
# speedup vs baseline: 1.0199x; 1.0199x over previous
"""Local causal (sliding-window) attention kernel for Trainium2, SPMD over 8 cores.

Problem: states [4, 4096, 1024] f32; q/k/v = states @ W*.T + b*; each query t
attends keys t-8..t (window=8), softmax over valid positions, out = attn @ v.

Sharding: data-parallel, 8 shards = 4 batches x 2 sequence halves (2048 queries
each). Each shard's states arrive pre-transposed and chunk-packed as
[128, 8, 2056] with an 8-col halo at the sequence start (zeros for the first
half; real previous-half tokens for the second half).

Score reformulation (saves one full GEMM): q.k = x_t^T A x_k + u[k] + const
with A = (Wq/sqrt(H))^T Wk precomputed on host. The device computes
Y = A @ X (one GEMM); scores come out TRANSPOSED (keys on partitions):
S^T_i = Y[:, frame_i]^T @ X[:, queries_i], which feeds softmax along the
partition dim with no transposes: exp bias = u[key] (per-partition), band
mask applied multiplicatively after exp, row-sum via a PE matmul against a
ones column, and P^T is directly the PV lhsT.

Tiling: 17 full tiles of 120 queries + 1 tail tile of 8. Each 120-query tile's
9-key windows span exactly 128 keys -> one sliding V frame per tile, so PV is
2 matmuls (plus the N=1 rowsum). V frames are recomputed on the 8-col overlap
(+6% V GEMM); the tail tile's 16-key V frame comes from the host (vtail).

Schedule: PE warm-up dummies during the initial weight DMA (HAM), Y GEMM
first (5 chunks, first small to shorten the DMA critical path), then a
per-frame software pipeline V_i | S^T_i | PV_{i-1} so the tail is one chain.
Inputs ride two HWDGE rings (x on sync, a/wv on scalar); outputs on gpsimd.
"""

import numpy as np
import ml_dtypes

import concourse.bacc as bacc
import concourse.mybir as mybir
import concourse.tile as tile
from concourse.bass_utils import run_bass_kernel_spmd

B, T, H = 4, 4096, 1024
NCORES = 8
TC = T // 2            # queries per core
HALO = 8               # window size
TH = TC + HALO         # x cols incl. halo
QT = 120               # queries per full tile (window spans exactly 128 keys)
NFT = 17               # full tiles; tail tile has TC - 17*120 = 8 queries
NTILE = NFT + 1
NQ_TAIL = TC - QT * NFT            # 8
NK_TAIL = NQ_TAIL + HALO           # 16
HC = H // 128          # 128-row chunks of H
NWARM = 18             # HAM warm-up dummy matmuls
# Y GEMM x-col chunks; first small so PE can start after ~2.5MB of DMA
YCHUNKS = [(8, 264), (264, 776), (776, 1288), (1288, 1800), (1800, 2056)]
XSEGS = [(0, 264), (264, 776), (776, 1288), (1288, 1800), (1800, 2056)]
F32 = mybir.dt.float32
BF16 = mybir.dt.bfloat16
BF = ml_dtypes.bfloat16
AF = mybir.ActivationFunctionType

_cache = {}


def _emit(nc, tc, aps, pools):
    (xs_d, a_d, wv_d, bv_d, m0_d, mr_d, u2_d, yh_d, vt_d, out_d) = aps
    consts, xw, acts, attn, psY, psS, psO, psR = pools

    warm = consts.tile([128, 512], BF16, tag="warm", name="warm")
    ones_t = consts.tile([128, 1], BF16, tag="ones", name="ones_t")
    bv_t = consts.tile([128, H], BF16, tag="bv", name="bv_t")
    m0_t = consts.tile([128, QT], BF16, tag="m0", name="m0_t")
    mr_t = consts.tile([128, QT], BF16, tag="mr", name="mr_t")
    u2_t = consts.tile([128, NTILE], F32, tag="u2", name="u2_t")
    vtail_t = consts.tile([NK_TAIL, H], BF16, tag="vtail", name="vtail_t")

    x_all = xw.tile([128, HC, TH], BF16, tag="x", name="x_all")
    a_all = xw.tile([128, HC, H], BF16, tag="a", name="a_all")
    wv_all = xw.tile([128, HC, H], BF16, tag="wv", name="wv_all")
    y_all = acts.tile([128, HC, TH], BF16, tag="y", name="y_all")
    vt = [acts.tile([128, H], BF16, tag=f"v{i}", name=f"v{i}")
          for i in range(NFT)]

    # ---- DMA issue. Critical path (xseg0 + a, in arrival-chased chunks) on
    # the scalar ring, which is otherwise idle early; everything else on the
    # sync ring, explicitly HELD until a lands so it doesn't steal HBM
    # bandwidth from the critical load. Outputs also go on sync (late).
    from concourse.bass import _add_dep_helper

    def xdma(s):
        lo, hi = XSEGS[s]
        return nc.sync.dma_start(x_all[:, :, lo:hi], xs_d[s][:])

    nc.scalar.dma_start(x_all[:, :, XSEGS[0][0]:XSEGS[0][1]], xs_d[0][:])
    ia = None
    for k in range(4):
        ia = nc.scalar.dma_start(a_all[:, 2 * k:2 * k + 2, :],
                                 a_d[:, 2 * k:2 * k + 2, :])
    ix1 = xdma(1)
    _add_dep_helper(getattr(ix1, "ins", ix1), getattr(ia, "ins", ia),
                    sync=True, reason="hold sync ring until a lands")
    xdma(2)
    xdma(3)
    xdma(4)
    nc.sync.dma_start(wv_all[:], wv_d[:])
    nc.sync.dma_start(y_all[:, :, 0:HALO], yh_d[:])
    nc.sync.dma_start(u2_t[:], u2_d[:])
    nc.sync.dma_start(vtail_t[:], vt_d[:])
    nc.sync.dma_start(bv_t[:], bv_d[:])
    nc.sync.dma_start(m0_t[:], m0_d[:])
    nc.sync.dma_start(mr_t[:], mr_d[:])

    # ---- PE warm-up on a zeroed tile while weights stream in (HAM) ----
    nc.vector.memset(warm[:], 0.0)
    nc.vector.memset(ones_t[:], 1.0)
    for _ in range(NWARM):
        ps = psY.tile([128, 512], F32, tag="ps", name="ps_warm")
        nc.tensor.matmul(ps[:], warm[:, 0:128], warm[:], start=True, stop=True)

    # ---- Y = A @ X over all x cols (halo cols from host) ----
    for ci, (lo, hi) in enumerate(YCHUNKS):
        for hc in range(HC):
            ps = psY.tile([128, hi - lo], F32, tag="ps", name="ps_y")
            for c in range(HC):
                nc.tensor.matmul(ps[:], a_all[:, c, hc * 128:(hc + 1) * 128],
                                 x_all[:, c, lo:hi],
                                 start=(c == 0), stop=(c == HC - 1))
            if hc % 2 == 0:
                nc.scalar.copy(y_all[:, hc, lo:hi], ps[:])
            else:
                nc.vector.tensor_copy(y_all[:, hc, lo:hi], ps[:])

    # ---- V frames + attention, software-pipelined per frame ----
    pm_tiles = {}
    rq = [QT] * NFT + [NQ_TAIL]
    rk = [128] * NFT + [NK_TAIL]

    def emit_v(i):
        for hh in range(2):
            ps = psY.tile([128, 512], F32, tag="ps", name="ps_v")
            for c in range(HC):
                nc.tensor.matmul(ps[:], x_all[:, c, QT * i: QT * i + 128],
                                 wv_all[:, c, hh * 512:(hh + 1) * 512],
                                 start=(c == 0), stop=(c == HC - 1))
            nc.vector.tensor_add(vt[i][:, hh * 512:(hh + 1) * 512], ps[:],
                                 bv_t[:, hh * 512:(hh + 1) * 512])

    def emit_s(i):
        nq, nk, f0 = rq[i], rk[i], QT * i
        s_ps = psS.tile([128, QT], F32, tag="s", name="s_ps")
        for c in range(HC):
            nc.tensor.matmul(s_ps[:nk, :nq], y_all[:, c, f0:f0 + nk],
                             x_all[:, c, f0 + HALO:f0 + HALO + nq],
                             start=(c == 0), stop=(c == HC - 1))
        p = attn.tile([128, QT], BF16, tag="p", name="p")
        nc.scalar.activation(p[:nk, :nq], s_ps[:nk, :nq], AF.Exp,
                             bias=u2_t[0:nk, i:i + 1], scale=1.0)
        pm = attn.tile([128, QT], BF16, tag="pm", name="pm")
        mask = m0_t if i == 0 else mr_t
        nc.vector.tensor_mul(pm[:nk, :nq], p[:nk, :nq], mask[0:nk, 0:nq])
        pm_tiles[i] = pm

    def emit_pv(i):
        nq, nk = rq[i], rk[i]
        pm = pm_tiles.pop(i)
        vsrc = vt[i] if i < NFT else vtail_t
        rs_ps = psR.tile([QT, 1], F32, tag="rs", name="rs_ps")
        nc.tensor.matmul(rs_ps[:nq, :], pm[:nk, :nq], ones_t[0:nk, :],
                         start=True, stop=True)
        rinv = attn.tile([QT, 1], F32, tag="ri", name="rinv")
        nc.vector.reciprocal(rinv[:nq, :], rs_ps[:nq, :])
        out_sb = attn.tile([QT, H], F32, tag="osb", name="out_sb")
        for hh in range(2):
            o_ps = psO.tile([QT, 512], F32, tag="o", name="o_ps")
            nc.tensor.matmul(o_ps[:nq, :], pm[:nk, :nq],
                             vsrc[0:nk, hh * 512:(hh + 1) * 512],
                             start=True, stop=True)
            if hh == 0:
                nc.scalar.activation(out_sb[:nq, 0:512], o_ps[:nq, :],
                                     AF.Copy, bias=0.0, scale=rinv[:nq, :])
                nc.sync.dma_start(out_d[QT * i: QT * i + nq, 0:512],
                                  out_sb[:nq, 0:512])
            else:
                nc.vector.tensor_scalar_mul(out_sb[:nq, 512:H], o_ps[:nq, :],
                                            rinv[:nq, :])
                nc.sync.dma_start(out_d[QT * i: QT * i + nq, 512:H],
                                  out_sb[:nq, 512:H])

    # Tail tile first: it needs only Y + vtail (no V frame), so its whole
    # chain clears early and the kernel tail is just tile 16's chain.
    emit_s(NTILE - 1)
    emit_pv(NTILE - 1)
    for i in range(NFT):
        emit_v(i)
        emit_s(i)
        if i >= 1:
            emit_pv(i - 1)
    emit_pv(NFT - 1)


def _build(loop_reps=None, trace_sim=False):
    key = ("nc", loop_reps, trace_sim)
    if key in _cache:
        return _cache[key]
    nc = bacc.Bacc("TRN2", target_bir_lowering=False, debug=False,
                   num_devices=NCORES)

    aps = (
        [nc.dram_tensor(f"x{s}", [128, HC, hi - lo], BF16,
                        kind="ExternalInput").ap()
         for s, (lo, hi) in enumerate(XSEGS)],
        nc.dram_tensor("a", [128, HC, H], BF16, kind="ExternalInput").ap(),
        nc.dram_tensor("wv", [128, HC, H], BF16, kind="ExternalInput").ap(),
        nc.dram_tensor("bv", [128, H], BF16, kind="ExternalInput").ap(),
        nc.dram_tensor("m0", [128, QT], BF16, kind="ExternalInput").ap(),
        nc.dram_tensor("mr", [128, QT], BF16, kind="ExternalInput").ap(),
        nc.dram_tensor("u2", [128, NTILE], F32, kind="ExternalInput").ap(),
        nc.dram_tensor("yhalo", [128, HC, HALO], BF16,
                       kind="ExternalInput").ap(),
        nc.dram_tensor("vtail", [NK_TAIL, H], BF16, kind="ExternalInput").ap(),
        nc.dram_tensor("out", [TC, H], F32, kind="ExternalOutput").ap(),
    )

    with tile.TileContext(nc, trace_sim=trace_sim) as tc:
        with (
            tc.tile_pool(name="consts", bufs=1) as consts,
            tc.tile_pool(name="xw", bufs=1) as xw,
            tc.tile_pool(name="acts", bufs=1) as acts,
            tc.tile_pool(name="attn", bufs=3) as attn,
            tc.tile_pool(name="psY", bufs=3, space="PSUM") as psY,
            tc.tile_pool(name="psS", bufs=2, space="PSUM") as psS,
            tc.tile_pool(name="psO", bufs=2, space="PSUM") as psO,
            tc.tile_pool(name="psR", bufs=1, space="PSUM") as psR,
        ):
            pools = (consts, xw, acts, attn, psY, psS, psO, psR)
            if loop_reps:
                with tc.For_i(0, loop_reps, 1):
                    _emit(nc, tc, aps, pools)
            else:
                _emit(nc, tc, aps, pools)

    nc.compile()
    _cache[key] = nc
    return nc


def _pack(m):
    """[128*HC, W] row-chunked -> [128, HC, W] (partition-major packing)."""
    w = m.shape[1]
    return np.ascontiguousarray(
        m.reshape(HC, 128, w).transpose(1, 0, 2))


def _host_inputs(states, Wq, bq, Wk, bk, Wv, bv):
    """Shared (per-run) host-side tensor prep."""
    scale = 1.0 / np.sqrt(H)
    Wq = np.asarray(Wq, np.float32)
    Wk = np.asarray(Wk, np.float32)
    Wv = np.asarray(Wv, np.float32)
    bq = np.asarray(bq, np.float32)
    bv = np.asarray(bv, np.float32)
    Wqs = Wq * scale
    # A = Wqs.T @ Wk ; device lhsT layout needs A.T = Wk.T @ Wqs
    at_h = np.ascontiguousarray(Wk.T @ Wqs).astype(BF)
    # per-key rank-1 vector; per-query term and constants cancel in softmax
    wt_h = Wk.T @ (bq * scale)
    wv_h = np.ascontiguousarray(Wv.T).astype(BF)
    a_p = _pack(at_h.astype(BF))
    wv_p = _pack(wv_h.astype(BF))
    bv_h = np.ascontiguousarray(np.broadcast_to(bv, (128, H))).astype(BF)
    k = np.arange(128)[:, None]
    t = np.arange(QT)[None, :]
    band = (k >= t) & (k <= t + HALO)
    mr_h = band.astype(BF)
    m0_h = (band & (k >= HALO)).astype(BF)
    return at_h, wt_h, wv_p, bv_h, m0_h, mr_h, a_p, bv


def _shard_maps(states, hosts):
    at_h, wt_h, wv_p, bv_h, m0_h, mr_h, a_p, bv = hosts
    a_f = at_h.astype(np.float32)      # A.T in bf16 precision
    wv_f = wv_p.transpose(1, 0, 2).reshape(H, H).astype(np.float32)  # Wv.T
    in_maps = []
    for i in range(NCORES):
        b, hf = i // 2, i % 2
        xs = np.zeros((TH, H), np.float32)
        if hf == 0:
            xs[HALO:] = states[b, 0:TC]
        else:
            xs[:] = states[b, TC - HALO: 2 * TC]
        x_h = np.ascontiguousarray(xs.T).astype(BF)   # [H, TH]
        x_f = x_h.astype(np.float32)
        u_full = wt_h @ x_f                            # [TH]
        u2 = np.zeros((128, NTILE), np.float32)
        for j in range(NFT):
            u2[:, j] = u_full[QT * j: QT * j + 128]
        u2[:NK_TAIL, NFT] = u_full[QT * NFT: QT * NFT + NK_TAIL]
        yh = (a_f.T @ x_f[:, :HALO])                   # [H, 8] = A @ x_halo
        vtail_h = (x_f[:, QT * NFT:].T @ wv_f + bv).astype(BF)  # [16, H]
        im = {
            "a": a_p, "wv": wv_p, "bv": bv_h,
            "m0": (m0_h if hf == 0 else mr_h), "mr": mr_h,
            "u2": u2, "yhalo": _pack(yh.astype(BF)), "vtail": vtail_h,
        }
        for s, (lo, hi) in enumerate(XSEGS):
            im[f"x{s}"] = _pack(x_h[:, lo:hi])
        in_maps.append(im)
    return in_maps


def kernel(states, Wq, bq, Wk, bk, Wv, bv, window):
    assert int(window) == HALO
    states = np.asarray(states, np.float32)
    nc = _build()
    hosts = _host_inputs(states, Wq, bq, Wk, bk, Wv, bv)
    in_maps = _shard_maps(states, hosts)
    res = run_bass_kernel_spmd(nc, in_maps, list(range(NCORES)))
    out = np.empty((B, T, H), np.float32)
    for i in range(NCORES):
        b, hf = i // 2, i % 2
        out[b, hf * TC:(hf + 1) * TC] = res.results[i]["out"]
    return out


# revision 16
# speedup vs baseline: 1.1015x; 1.0800x over previous
"""Local causal (sliding-window) attention kernel for Trainium2, SPMD over 8 cores.

Problem: states [4, 4096, 1024] f32; q/k/v = states @ W*.T + b*; each query t
attends keys t-8..t (window=8), softmax over valid positions, out = attn @ v.

Sharding: data-parallel, 8 shards = 4 batches x 2 sequence halves (2048 queries
each). Each shard's states arrive pre-transposed and chunk-packed as
[128, 8, 2056] with an 8-col halo at the sequence start (zeros for the first
half; real previous-half tokens for the second half).

Score reformulation (saves one full GEMM): q.k = x_t^T A x_k + u[k] + const
with A = (Wq/sqrt(H))^T Wk precomputed on host. The device computes
Y = A @ X (one GEMM); scores come out TRANSPOSED (keys on partitions):
S^T_i = Y[:, frame_i]^T @ X[:, queries_i], which feeds softmax along the
partition dim with no transposes: exp bias = u[key] (per-partition), band
mask applied multiplicatively after exp, row-sum via a PE matmul against a
ones column, and P^T is directly the PV lhsT.

Tiling: 17 full tiles of 120 queries + 1 tail tile of 8. Each 120-query tile's
9-key windows span exactly 128 keys -> one sliding V frame per tile, so PV is
2 matmuls (plus the N=1 rowsum). V frames are recomputed on the 8-col overlap
(+6% V GEMM); the tail tile's 16-key V frame comes from the host (vtail).

Schedule: PE warm-up dummies during the initial weight DMA (HAM), Y GEMM
first (5 chunks, first small to shorten the DMA critical path), then a
per-frame software pipeline V_i | S^T_i | PV_{i-1} so the tail is one chain.
Inputs ride two HWDGE rings (x on sync, a/wv on scalar); outputs on gpsimd.
"""

import numpy as np
import ml_dtypes

import concourse.bacc as bacc
import concourse.mybir as mybir
import concourse.tile as tile
from concourse.bass_utils import run_bass_kernel_spmd

B, T, H = 4, 4096, 1024
NCORES = 8
TC = T // 2            # queries per core
HALO = 8               # window size
TH = TC + HALO         # x cols incl. halo
QT = 120               # queries per full tile (window spans exactly 128 keys)
NFT = 17               # full tiles; tail tile has TC - 17*120 = 8 queries
NTILE = NFT + 1
NQ_TAIL = TC - QT * NFT            # 8
NK_TAIL = NQ_TAIL + HALO           # 16
HC = H // 128          # 128-row chunks of H
NWARM = 18             # HAM warm-up dummy matmuls
# Y GEMM x-col chunks; first small so PE can start after ~2.5MB of DMA
YCHUNKS = [(8, 264), (264, 776), (776, 1288), (1288, 1800), (1800, 2056)]
XSEGS = [(0, 264), (264, 776), (776, 1288), (1288, 1800), (1800, 2056)]
F32 = mybir.dt.float32
BF16 = mybir.dt.bfloat16
BF = ml_dtypes.bfloat16
AF = mybir.ActivationFunctionType

_cache = {}


def _emit(nc, tc, aps, pools):
    (xs_d, a_d, wv_d, bv_d, m0_d, mr_d, u2_d, yh_d, vt_d, out_d) = aps
    consts, xw, acts, attn, psY, psS, psO, psR = pools

    warm = consts.tile([128, 512], BF16, tag="warm", name="warm")
    ones_t = consts.tile([128, 1], BF16, tag="ones", name="ones_t")
    bv_t = consts.tile([128, H], BF16, tag="bv", name="bv_t")
    m0_t = consts.tile([128, QT], BF16, tag="m0", name="m0_t")
    mr_t = consts.tile([128, QT], BF16, tag="mr", name="mr_t")
    u2_t = consts.tile([128, NTILE], F32, tag="u2", name="u2_t")
    vtail_t = consts.tile([NK_TAIL, H], BF16, tag="vtail", name="vtail_t")

    x_all = xw.tile([128, HC, TH], BF16, tag="x", name="x_all")
    a_all = xw.tile([128, HC, H], BF16, tag="a", name="a_all")
    wv_all = xw.tile([128, HC, H], BF16, tag="wv", name="wv_all")
    y_all = acts.tile([128, HC, TH], BF16, tag="y", name="y_all")
    vt = [acts.tile([128, H], BF16, tag=f"v{i}", name=f"v{i}")
          for i in range(NFT)]

    # ---- DMA issue. Critical path (xseg0 + a, in arrival-chased chunks) on
    # the scalar ring, which is otherwise idle early; everything else on the
    # sync ring, explicitly HELD until a lands so it doesn't steal HBM
    # bandwidth from the critical load. Outputs also go on sync (late).
    from concourse.bass import _add_dep_helper

    def xdma(eng, s):
        lo, hi = XSEGS[s]
        return eng.dma_start(x_all[:, :, lo:hi], xs_d[s][:])

    xdma(nc.scalar, 0)
    for k in range(4):
        nc.scalar.dma_start(a_all[:, 2 * k:2 * k + 2, :],
                            a_d[:, 2 * k:2 * k + 2, :])
    xdma(nc.scalar, 1)
    ix2 = xdma(nc.scalar, 2)
    gate = getattr(ix2, "ins", ix2)

    def held(inst):
        _add_dep_helper(getattr(inst, "ins", inst), gate, sync=True,
                        reason="hold sync-ring inputs off the critical load")
    held(xdma(nc.sync, 3))
    held(xdma(nc.sync, 4))
    held(nc.sync.dma_start(wv_all[:], wv_d[:]))
    held(nc.sync.dma_start(y_all[:, :, 0:HALO], yh_d[:]))
    held(nc.sync.dma_start(u2_t[:], u2_d[:]))
    held(nc.sync.dma_start(vtail_t[:], vt_d[:]))
    held(nc.sync.dma_start(bv_t[:], bv_d[:]))
    held(nc.sync.dma_start(m0_t[:], m0_d[:]))
    held(nc.sync.dma_start(mr_t[:], mr_d[:]))

    # ---- PE warm-up on a zeroed tile while weights stream in (HAM) ----
    nc.vector.memset(warm[:], 0.0)
    nc.vector.memset(ones_t[:], 1.0)
    for _ in range(NWARM):
        ps = psY.tile([128, 512], F32, tag="ps", name="ps_warm")
        nc.tensor.matmul(ps[:], warm[:, 0:128], warm[:], start=True, stop=True)

    # ---- Y = A @ X over all x cols (halo cols from host) ----
    for ci, (lo, hi) in enumerate(YCHUNKS):
        for hc in range(HC):
            ps = psY.tile([128, hi - lo], F32, tag="ps", name="ps_y")
            for c in range(HC):
                nc.tensor.matmul(ps[:], a_all[:, c, hc * 128:(hc + 1) * 128],
                                 x_all[:, c, lo:hi],
                                 start=(c == 0), stop=(c == HC - 1))
            nc.vector.tensor_copy(y_all[:, hc, lo:hi], ps[:])

    # ---- V frames + attention, software-pipelined per frame ----
    pm_tiles = {}
    rq = [QT] * NFT + [NQ_TAIL]
    rk = [128] * NFT + [NK_TAIL]

    def emit_v(i):
        for hh in range(2):
            ps = psY.tile([128, 512], F32, tag="ps", name="ps_v")
            for c in range(HC):
                nc.tensor.matmul(ps[:], x_all[:, c, QT * i: QT * i + 128],
                                 wv_all[:, c, hh * 512:(hh + 1) * 512],
                                 start=(c == 0), stop=(c == HC - 1))
            nc.vector.tensor_add(vt[i][:, hh * 512:(hh + 1) * 512], ps[:],
                                 bv_t[:, hh * 512:(hh + 1) * 512])

    def emit_s(i):
        nq, nk, f0 = rq[i], rk[i], QT * i
        s_ps = psS.tile([128, QT], F32, tag="s", name="s_ps")
        for c in range(HC):
            nc.tensor.matmul(s_ps[:nk, :nq], y_all[:, c, f0:f0 + nk],
                             x_all[:, c, f0 + HALO:f0 + HALO + nq],
                             start=(c == 0), stop=(c == HC - 1))
        p = attn.tile([128, QT], BF16, tag="p", name="p")
        nc.scalar.activation(p[:nk, :nq], s_ps[:nk, :nq], AF.Exp,
                             bias=u2_t[0:nk, i:i + 1], scale=1.0)
        pm = attn.tile([128, QT], BF16, tag="pm", name="pm")
        mask = m0_t if i == 0 else mr_t
        nc.vector.tensor_mul(pm[:nk, :nq], p[:nk, :nq], mask[0:nk, 0:nq])
        pm_tiles[i] = pm

    def emit_pv(i):
        nq, nk = rq[i], rk[i]
        pm = pm_tiles.pop(i)
        vsrc = vt[i] if i < NFT else vtail_t
        rs_ps = psR.tile([QT, 1], F32, tag="rs", name="rs_ps")
        nc.tensor.matmul(rs_ps[:nq, :], pm[:nk, :nq], ones_t[0:nk, :],
                         start=True, stop=True)
        rinv = attn.tile([QT, 1], F32, tag="ri", name="rinv")
        nc.vector.reciprocal(rinv[:nq, :], rs_ps[:nq, :])
        out_sb = attn.tile([QT, H], F32, tag="osb", name="out_sb")
        for hh in range(2):
            o_ps = psO.tile([QT, 512], F32, tag="o", name="o_ps")
            nc.tensor.matmul(o_ps[:nq, :], pm[:nk, :nq],
                             vsrc[0:nk, hh * 512:(hh + 1) * 512],
                             start=True, stop=True)
            if hh == 0:
                nc.scalar.activation(out_sb[:nq, 0:512], o_ps[:nq, :],
                                     AF.Copy, bias=0.0, scale=rinv[:nq, :])
                nc.sync.dma_start(out_d[QT * i: QT * i + nq, 0:512],
                                  out_sb[:nq, 0:512])
            else:
                nc.vector.tensor_scalar_mul(out_sb[:nq, 512:H], o_ps[:nq, :],
                                            rinv[:nq, :])
                nc.sync.dma_start(out_d[QT * i: QT * i + nq, 512:H],
                                  out_sb[:nq, 512:H])

    # Tail tile first: it needs only Y + vtail (no V frame), so its whole
    # chain clears early and the kernel tail is just tile 16's chain.
    emit_s(NTILE - 1)
    emit_pv(NTILE - 1)
    for i in range(NFT):
        emit_v(i)
        emit_s(i)
        if i >= 1:
            emit_pv(i - 1)
    emit_pv(NFT - 1)


def _build(loop_reps=None, trace_sim=False):
    key = ("nc", loop_reps, trace_sim)
    if key in _cache:
        return _cache[key]
    nc = bacc.Bacc("TRN2", target_bir_lowering=False, debug=False,
                   num_devices=NCORES)

    aps = (
        [nc.dram_tensor(f"x{s}", [128, HC, hi - lo], BF16,
                        kind="ExternalInput").ap()
         for s, (lo, hi) in enumerate(XSEGS)],
        nc.dram_tensor("a", [128, HC, H], BF16, kind="ExternalInput").ap(),
        nc.dram_tensor("wv", [128, HC, H], BF16, kind="ExternalInput").ap(),
        nc.dram_tensor("bv", [128, H], BF16, kind="ExternalInput").ap(),
        nc.dram_tensor("m0", [128, QT], BF16, kind="ExternalInput").ap(),
        nc.dram_tensor("mr", [128, QT], BF16, kind="ExternalInput").ap(),
        nc.dram_tensor("u2", [128, NTILE], F32, kind="ExternalInput").ap(),
        nc.dram_tensor("yhalo", [128, HC, HALO], BF16,
                       kind="ExternalInput").ap(),
        nc.dram_tensor("vtail", [NK_TAIL, H], BF16, kind="ExternalInput").ap(),
        nc.dram_tensor("out", [TC, H], F32, kind="ExternalOutput").ap(),
    )

    with tile.TileContext(nc, trace_sim=trace_sim) as tc:
        with (
            tc.tile_pool(name="consts", bufs=1) as consts,
            tc.tile_pool(name="xw", bufs=1) as xw,
            tc.tile_pool(name="acts", bufs=1) as acts,
            tc.tile_pool(name="attn", bufs=3) as attn,
            tc.tile_pool(name="psY", bufs=3, space="PSUM") as psY,
            tc.tile_pool(name="psS", bufs=2, space="PSUM") as psS,
            tc.tile_pool(name="psO", bufs=2, space="PSUM") as psO,
            tc.tile_pool(name="psR", bufs=1, space="PSUM") as psR,
        ):
            pools = (consts, xw, acts, attn, psY, psS, psO, psR)
            if loop_reps:
                with tc.For_i(0, loop_reps, 1):
                    _emit(nc, tc, aps, pools)
            else:
                _emit(nc, tc, aps, pools)

    nc.compile()
    _cache[key] = nc
    return nc


def _pack(m):
    """[128*HC, W] row-chunked -> [128, HC, W] (partition-major packing)."""
    w = m.shape[1]
    return np.ascontiguousarray(
        m.reshape(HC, 128, w).transpose(1, 0, 2))


def _host_inputs(states, Wq, bq, Wk, bk, Wv, bv):
    """Shared (per-run) host-side tensor prep."""
    scale = 1.0 / np.sqrt(H)
    Wq = np.asarray(Wq, np.float32)
    Wk = np.asarray(Wk, np.float32)
    Wv = np.asarray(Wv, np.float32)
    bq = np.asarray(bq, np.float32)
    bv = np.asarray(bv, np.float32)
    Wqs = Wq * scale
    # A = Wqs.T @ Wk ; device lhsT layout needs A.T = Wk.T @ Wqs
    at_h = np.ascontiguousarray(Wk.T @ Wqs).astype(BF)
    # per-key rank-1 vector; per-query term and constants cancel in softmax
    wt_h = Wk.T @ (bq * scale)
    wv_h = np.ascontiguousarray(Wv.T).astype(BF)
    a_p = _pack(at_h.astype(BF))
    wv_p = _pack(wv_h.astype(BF))
    bv_h = np.ascontiguousarray(np.broadcast_to(bv, (128, H))).astype(BF)
    k = np.arange(128)[:, None]
    t = np.arange(QT)[None, :]
    band = (k >= t) & (k <= t + HALO)
    mr_h = band.astype(BF)
    m0_h = (band & (k >= HALO)).astype(BF)
    return at_h, wt_h, wv_p, bv_h, m0_h, mr_h, a_p, bv


def _shard_maps(states, hosts):
    at_h, wt_h, wv_p, bv_h, m0_h, mr_h, a_p, bv = hosts
    a_f = at_h.astype(np.float32)      # A.T in bf16 precision
    wv_f = wv_p.transpose(1, 0, 2).reshape(H, H).astype(np.float32)  # Wv.T
    in_maps = []
    for i in range(NCORES):
        b, hf = i // 2, i % 2
        xs = np.zeros((TH, H), np.float32)
        if hf == 0:
            xs[HALO:] = states[b, 0:TC]
        else:
            xs[:] = states[b, TC - HALO: 2 * TC]
        x_h = np.ascontiguousarray(xs.T).astype(BF)   # [H, TH]
        x_f = x_h.astype(np.float32)
        u_full = wt_h @ x_f                            # [TH]
        u2 = np.zeros((128, NTILE), np.float32)
        for j in range(NFT):
            u2[:, j] = u_full[QT * j: QT * j + 128]
        u2[:NK_TAIL, NFT] = u_full[QT * NFT: QT * NFT + NK_TAIL]
        yh = (a_f.T @ x_f[:, :HALO])                   # [H, 8] = A @ x_halo
        vtail_h = (x_f[:, QT * NFT:].T @ wv_f + bv).astype(BF)  # [16, H]
        im = {
            "a": a_p, "wv": wv_p, "bv": bv_h,
            "m0": (m0_h if hf == 0 else mr_h), "mr": mr_h,
            "u2": u2, "yhalo": _pack(yh.astype(BF)), "vtail": vtail_h,
        }
        for s, (lo, hi) in enumerate(XSEGS):
            im[f"x{s}"] = _pack(x_h[:, lo:hi])
        in_maps.append(im)
    return in_maps


def kernel(states, Wq, bq, Wk, bk, Wv, bv, window):
    assert int(window) == HALO
    states = np.asarray(states, np.float32)
    nc = _build()
    hosts = _host_inputs(states, Wq, bq, Wk, bk, Wv, bv)
    in_maps = _shard_maps(states, hosts)
    res = run_bass_kernel_spmd(nc, in_maps, list(range(NCORES)))
    out = np.empty((B, T, H), np.float32)
    for i in range(NCORES):
        b, hf = i // 2, i % 2
        out[b, hf * TC:(hf + 1) * TC] = res.results[i]["out"]
    return out


# revision 19
# speedup vs baseline: 1.1152x; 1.0125x over previous
"""Local causal (sliding-window) attention kernel for Trainium2, SPMD over 8 cores.

Problem: states [4, 4096, 1024] f32; q/k/v = states @ W*.T + b*; each query t
attends keys t-8..t (window=8), softmax over valid positions, out = attn @ v.

Sharding: data-parallel, 8 shards = 4 batches x 2 sequence halves (2048 queries
each). Each shard's states arrive pre-transposed and chunk-packed as
[128, 8, 2056] with an 8-col halo at the sequence start (zeros for the first
half; real previous-half tokens for the second half).

Score reformulation (saves one full GEMM): q.k = x_t^T A x_k + u[k] + const
with A = (Wq/sqrt(H))^T Wk precomputed on host. The device computes
Y = A @ X (one GEMM); scores come out TRANSPOSED (keys on partitions):
S^T_i = Y[:, frame_i]^T @ X[:, queries_i], which feeds softmax along the
partition dim with no transposes: exp bias = u[key] (per-partition), band
mask applied multiplicatively after exp, row-sum via a PE matmul against a
ones column, and P^T is directly the PV lhsT.

Tiling: 17 full tiles of 120 queries + 1 tail tile of 8. Each 120-query tile's
9-key windows span exactly 128 keys -> one sliding V frame per tile, so PV is
2 matmuls (plus the N=1 rowsum). V frames are recomputed on the 8-col overlap
(+6% V GEMM); the tail tile's 16-key V frame comes from the host (vtail).

Schedule: PE warm-up dummies during the initial weight DMA (HAM), Y GEMM
first (5 chunks, first small to shorten the DMA critical path), then a
per-frame software pipeline V_i | S^T_i | PV_{i-1} so the tail is one chain.
Inputs ride two HWDGE rings (x on sync, a/wv on scalar); outputs on gpsimd.
"""

import numpy as np
import ml_dtypes

import concourse.bacc as bacc
import concourse.mybir as mybir
import concourse.tile as tile
from concourse.bass_utils import run_bass_kernel_spmd

B, T, H = 4, 4096, 1024
NCORES = 8
TC = T // 2            # queries per core
HALO = 8               # window size
TH = TC + HALO         # x cols incl. halo
QT = 120               # queries per full tile (window spans exactly 128 keys)
NFT = 17               # full tiles; tail tile has TC - 17*120 = 8 queries
NTILE = NFT + 1
NQ_TAIL = TC - QT * NFT            # 8
NK_TAIL = NQ_TAIL + HALO           # 16
HC = H // 128          # 128-row chunks of H
NWARM = 18             # HAM warm-up dummy matmuls
# Y GEMM x-col chunks; first small so PE can start after ~2.5MB of DMA
YCHUNKS = [(8, 264), (264, 776), (776, 1288), (1288, 1800), (1800, 2056)]
XSEGS = [(0, 264), (264, 776), (776, 1288), (1288, 1800), (1800, 2056)]
F32 = mybir.dt.float32
BF16 = mybir.dt.bfloat16
BF = ml_dtypes.bfloat16
AF = mybir.ActivationFunctionType

_cache = {}


def _emit(nc, tc, aps, pools):
    (xs_d, a_d, wv_d, bv_d, m0_d, mr_d, u2_d, yh_d, vt_d, out_d) = aps
    consts, xw, acts, attn, psY, psS, psO, psR = pools

    warm = consts.tile([128, 512], BF16, tag="warm", name="warm")
    ones_t = consts.tile([128, 1], BF16, tag="ones", name="ones_t")
    bv_t = consts.tile([128, H], BF16, tag="bv", name="bv_t")
    m0_t = consts.tile([128, QT], BF16, tag="m0", name="m0_t")
    mr_t = consts.tile([128, QT], BF16, tag="mr", name="mr_t")
    u2_t = consts.tile([128, NTILE], F32, tag="u2", name="u2_t")
    vtail_t = consts.tile([NK_TAIL, H], BF16, tag="vtail", name="vtail_t")

    x_all = xw.tile([128, HC, TH], BF16, tag="x", name="x_all")
    a_all = xw.tile([128, HC, H], BF16, tag="a", name="a_all")
    wv_all = xw.tile([128, HC, H], BF16, tag="wv", name="wv_all")
    y_all = acts.tile([128, HC, TH], BF16, tag="y", name="y_all")
    vt = [acts.tile([128, H], BF16, tag=f"v{i}", name=f"v{i}")
          for i in range(NFT)]

    # ---- DMA issue. Critical path (xseg0 + a, in arrival-chased chunks) on
    # the scalar ring, which is otherwise idle early; everything else on the
    # sync ring, explicitly HELD until a lands so it doesn't steal HBM
    # bandwidth from the critical load. Outputs also go on sync (late).
    from concourse.bass import _add_dep_helper

    def xdma(eng, s):
        lo, hi = XSEGS[s]
        return eng.dma_start(x_all[:, :, lo:hi], xs_d[s][:])

    xdma(nc.scalar, 0)
    for k in range(4):
        nc.scalar.dma_start(a_all[:, 2 * k:2 * k + 2, :],
                            a_d[:, 2 * k:2 * k + 2, :])
    xdma(nc.scalar, 1)
    ix2 = xdma(nc.scalar, 2)
    gate = getattr(ix2, "ins", ix2)

    def held(inst):
        _add_dep_helper(getattr(inst, "ins", inst), gate, sync=True,
                        reason="hold sync-ring inputs off the critical load")
    held(xdma(nc.sync, 3))
    held(xdma(nc.sync, 4))
    held(nc.sync.dma_start(wv_all[:], wv_d[:]))
    held(nc.sync.dma_start(y_all[:, :, 0:HALO], yh_d[:]))
    held(nc.sync.dma_start(u2_t[:], u2_d[:]))
    held(nc.sync.dma_start(vtail_t[:], vt_d[:]))
    held(nc.sync.dma_start(bv_t[:], bv_d[:]))
    held(nc.sync.dma_start(m0_t[:], m0_d[:]))
    held(nc.sync.dma_start(mr_t[:], mr_d[:]))

    # ---- PE warm-up on a zeroed tile while weights stream in (HAM) ----
    nc.vector.memset(warm[:], 0.0)
    nc.vector.memset(ones_t[:], 1.0)
    for _ in range(NWARM):
        ps = psY.tile([128, 512], F32, tag="ps", name="ps_warm")
        nc.tensor.matmul(ps[:], warm[:, 0:128], warm[:], start=True, stop=True)

    # ---- Y = A @ X over all x cols (halo cols from host) ----
    for ci, (lo, hi) in enumerate(YCHUNKS):
        for hc in range(HC):
            ps = psY.tile([128, hi - lo], F32, tag="ps", name="ps_y")
            for c in range(HC):
                nc.tensor.matmul(ps[:], a_all[:, c, hc * 128:(hc + 1) * 128],
                                 x_all[:, c, lo:hi],
                                 start=(c == 0), stop=(c == HC - 1))
            nc.vector.tensor_copy(y_all[:, hc, lo:hi], ps[:])

    # ---- V frames + attention, software-pipelined per frame ----
    pm_tiles = {}
    rq = [QT] * NFT + [NQ_TAIL]
    rk = [128] * NFT + [NK_TAIL]

    def emit_v(i):
        for hh in range(2):
            ps = psY.tile([128, 512], F32, tag="ps", name="ps_v")
            for c in range(HC):
                nc.tensor.matmul(ps[:], x_all[:, c, QT * i: QT * i + 128],
                                 wv_all[:, c, hh * 512:(hh + 1) * 512],
                                 start=(c == 0), stop=(c == HC - 1))
            nc.vector.tensor_add(vt[i][:, hh * 512:(hh + 1) * 512], ps[:],
                                 bv_t[:, hh * 512:(hh + 1) * 512])

    def emit_s(i):
        nq, nk, f0 = rq[i], rk[i], QT * i
        s_ps = psS.tile([128, QT], F32, tag="s", name="s_ps")
        for c in range(HC):
            nc.tensor.matmul(s_ps[:nk, :nq], y_all[:, c, f0:f0 + nk],
                             x_all[:, c, f0 + HALO:f0 + HALO + nq],
                             start=(c == 0), stop=(c == HC - 1))
        p = attn.tile([128, QT], BF16, tag="p", name="p")
        nc.scalar.activation(p[:nk, :nq], s_ps[:nk, :nq], AF.Exp,
                             bias=u2_t[0:nk, i:i + 1], scale=1.0)
        pm = attn.tile([128, QT], BF16, tag="pm", name="pm")
        mask = m0_t if i == 0 else mr_t
        nc.vector.tensor_mul(pm[:nk, :nq], p[:nk, :nq], mask[0:nk, 0:nq])
        pm_tiles[i] = pm

    def emit_pv(i):
        nq, nk = rq[i], rk[i]
        pm = pm_tiles.pop(i)
        vsrc = vt[i] if i < NFT else vtail_t
        rs_ps = psR.tile([QT, 1], F32, tag="rs", name="rs_ps")
        nc.tensor.matmul(rs_ps[:nq, :], pm[:nk, :nq], ones_t[0:nk, :],
                         start=True, stop=True)
        rinv = attn.tile([QT, 1], F32, tag="ri", name="rinv")
        nc.vector.reciprocal(rinv[:nq, :], rs_ps[:nq, :])
        out_sb = attn.tile([QT, H], F32, tag="osb", name="out_sb")
        for hh in range(2):
            o_ps = psO.tile([QT, 512], F32, tag="o", name="o_ps")
            nc.tensor.matmul(o_ps[:nq, :], pm[:nk, :nq],
                             vsrc[0:nk, hh * 512:(hh + 1) * 512],
                             start=True, stop=True)
            if hh == 0:
                nc.scalar.activation(out_sb[:nq, 0:512], o_ps[:nq, :],
                                     AF.Copy, bias=0.0, scale=rinv[:nq, :])
                nc.sync.dma_start(out_d[QT * i: QT * i + nq, 0:512],
                                  out_sb[:nq, 0:512])
            else:
                nc.vector.tensor_scalar_mul(out_sb[:nq, 512:H], o_ps[:nq, :],
                                            rinv[:nq, :])
                nc.scalar.dma_start(out_d[QT * i: QT * i + nq, 512:H],
                                    out_sb[:nq, 512:H])

    # Tail tile first: it needs only Y + vtail (no V frame), so its whole
    # chain clears early and the kernel tail is just tile 16's chain.
    # Within a frame, S^T before V: S^T only needs Y, so its softmax chain
    # resolves under V's matmul stream and PV_{i-1} never stalls PE.
    emit_s(NTILE - 1)
    emit_pv(NTILE - 1)
    for i in range(NFT):
        emit_s(i)
        emit_v(i)
        if i >= 1:
            emit_pv(i - 1)
    emit_pv(NFT - 1)


def _build(loop_reps=None, trace_sim=False):
    key = ("nc", loop_reps, trace_sim)
    if key in _cache:
        return _cache[key]
    nc = bacc.Bacc("TRN2", target_bir_lowering=False, debug=False,
                   num_devices=NCORES)

    aps = (
        [nc.dram_tensor(f"x{s}", [128, HC, hi - lo], BF16,
                        kind="ExternalInput").ap()
         for s, (lo, hi) in enumerate(XSEGS)],
        nc.dram_tensor("a", [128, HC, H], BF16, kind="ExternalInput").ap(),
        nc.dram_tensor("wv", [128, HC, H], BF16, kind="ExternalInput").ap(),
        nc.dram_tensor("bv", [128, H], BF16, kind="ExternalInput").ap(),
        nc.dram_tensor("m0", [128, QT], BF16, kind="ExternalInput").ap(),
        nc.dram_tensor("mr", [128, QT], BF16, kind="ExternalInput").ap(),
        nc.dram_tensor("u2", [128, NTILE], F32, kind="ExternalInput").ap(),
        nc.dram_tensor("yhalo", [128, HC, HALO], BF16,
                       kind="ExternalInput").ap(),
        nc.dram_tensor("vtail", [NK_TAIL, H], BF16, kind="ExternalInput").ap(),
        nc.dram_tensor("out", [TC, H], F32, kind="ExternalOutput").ap(),
    )

    with tile.TileContext(nc, trace_sim=trace_sim) as tc:
        with (
            tc.tile_pool(name="consts", bufs=1) as consts,
            tc.tile_pool(name="xw", bufs=1) as xw,
            tc.tile_pool(name="acts", bufs=1) as acts,
            tc.tile_pool(name="attn", bufs=3) as attn,
            tc.tile_pool(name="psY", bufs=4, space="PSUM") as psY,
            tc.tile_pool(name="psS", bufs=1, space="PSUM") as psS,
            tc.tile_pool(name="psO", bufs=2, space="PSUM") as psO,
            tc.tile_pool(name="psR", bufs=1, space="PSUM") as psR,
        ):
            pools = (consts, xw, acts, attn, psY, psS, psO, psR)
            if loop_reps:
                with tc.For_i(0, loop_reps, 1):
                    _emit(nc, tc, aps, pools)
            else:
                _emit(nc, tc, aps, pools)

    nc.compile()
    _cache[key] = nc
    return nc


def _pack(m):
    """[128*HC, W] row-chunked -> [128, HC, W] (partition-major packing)."""
    w = m.shape[1]
    return np.ascontiguousarray(
        m.reshape(HC, 128, w).transpose(1, 0, 2))


def _host_inputs(states, Wq, bq, Wk, bk, Wv, bv):
    """Shared (per-run) host-side tensor prep."""
    scale = 1.0 / np.sqrt(H)
    Wq = np.asarray(Wq, np.float32)
    Wk = np.asarray(Wk, np.float32)
    Wv = np.asarray(Wv, np.float32)
    bq = np.asarray(bq, np.float32)
    bv = np.asarray(bv, np.float32)
    Wqs = Wq * scale
    # A = Wqs.T @ Wk ; device lhsT layout needs A.T = Wk.T @ Wqs
    at_h = np.ascontiguousarray(Wk.T @ Wqs).astype(BF)
    # per-key rank-1 vector; per-query term and constants cancel in softmax
    wt_h = Wk.T @ (bq * scale)
    wv_h = np.ascontiguousarray(Wv.T).astype(BF)
    a_p = _pack(at_h.astype(BF))
    wv_p = _pack(wv_h.astype(BF))
    bv_h = np.ascontiguousarray(np.broadcast_to(bv, (128, H))).astype(BF)
    k = np.arange(128)[:, None]
    t = np.arange(QT)[None, :]
    band = (k >= t) & (k <= t + HALO)
    mr_h = band.astype(BF)
    m0_h = (band & (k >= HALO)).astype(BF)
    return at_h, wt_h, wv_p, bv_h, m0_h, mr_h, a_p, bv


def _shard_maps(states, hosts):
    at_h, wt_h, wv_p, bv_h, m0_h, mr_h, a_p, bv = hosts
    a_f = at_h.astype(np.float32)      # A.T in bf16 precision
    wv_f = wv_p.transpose(1, 0, 2).reshape(H, H).astype(np.float32)  # Wv.T
    in_maps = []
    for i in range(NCORES):
        b, hf = i // 2, i % 2
        xs = np.zeros((TH, H), np.float32)
        if hf == 0:
            xs[HALO:] = states[b, 0:TC]
        else:
            xs[:] = states[b, TC - HALO: 2 * TC]
        x_h = np.ascontiguousarray(xs.T).astype(BF)   # [H, TH]
        x_f = x_h.astype(np.float32)
        u_full = wt_h @ x_f                            # [TH]
        u2 = np.zeros((128, NTILE), np.float32)
        for j in range(NFT):
            u2[:, j] = u_full[QT * j: QT * j + 128]
        u2[:NK_TAIL, NFT] = u_full[QT * NFT: QT * NFT + NK_TAIL]
        yh = (a_f.T @ x_f[:, :HALO])                   # [H, 8] = A @ x_halo
        vtail_h = (x_f[:, QT * NFT:].T @ wv_f + bv).astype(BF)  # [16, H]
        im = {
            "a": a_p, "wv": wv_p, "bv": bv_h,
            "m0": (m0_h if hf == 0 else mr_h), "mr": mr_h,
            "u2": u2, "yhalo": _pack(yh.astype(BF)), "vtail": vtail_h,
        }
        for s, (lo, hi) in enumerate(XSEGS):
            im[f"x{s}"] = _pack(x_h[:, lo:hi])
        in_maps.append(im)
    return in_maps


def kernel(states, Wq, bq, Wk, bk, Wv, bv, window):
    assert int(window) == HALO
    states = np.asarray(states, np.float32)
    nc = _build()
    hosts = _host_inputs(states, Wq, bq, Wk, bk, Wv, bv)
    in_maps = _shard_maps(states, hosts)
    res = run_bass_kernel_spmd(nc, in_maps, list(range(NCORES)))
    out = np.empty((B, T, H), np.float32)
    for i in range(NCORES):
        b, hf = i // 2, i % 2
        out[b, hf * TC:(hf + 1) * TC] = res.results[i]["out"]
    return out
